# revision 12
# baseline (speedup 1.0000x reference)
"""Trainium2 Bass kernel for nn_Attention_50964081935360.

Single-query attention with a global-Frobenius-norm score scale:
  scores[b,s] = key[b,s,:] . query[b,:]
  denom      = ||key||_F  (over the WHOLE key tensor, all batches)
  p          = softmax(scores/denom) masked to s < seq_lens[b], renormalized
  out        = p[..., None] + 1e-15

Sharding: data-parallel over batch B=32 across 8 NeuronCores (4 batches per
core); one scalar AllReduce of the key-shard sum of squares, split in two
stages so the mesh latency hides under the stream.

v2 tail restructure (the v1 kernel lost ~45 us after the DMA stream ended):
  - The AR input reductions NEVER touch the DVE (which is saturated with
    affine_mul_reduce score columns until ~3 us after the stream ends).
    ssq column partial sums go ACT-Square+accum -> PE all-ones matmul
    (partition sum) -> ACT Copy+accum (column sum) -> cc_in DMA.
  - AR1 covers supertiles 0..13 and fires at ~77% of the stream; AR2
    covers 14..15 and fires ~2.5 us after the last key byte.
  - The exp epilogue runs on the AR1-only scale inv1 = rsqrt(g1) BEFORE
    AR2 lands; AR2's contribution is applied as an exact-to-fp32
    linearized correction E = E1*(1 + w*delta), delta = r*(-1/2 + 3r/8
    - 5r^2/16), r = g2/g1 (|w*delta| ~ 1e-4, cubic truncation ~1e-5 of
    inv -> ~1e-7 on p). Column sums of both E1*mask and E1*mask*w are
    pre-reduced, so the post-AR2 critical path is ~10 tiny DVE ops.
  - First and last supertiles stream in 4x 1 MiB chunks (subtile deps):
    the first AMR starts at ~10.5 us instead of ~22; the post-stream
    drain is one chunk (~3 us) instead of a full supertile.
  - masks/s_idx/q broadcast all sit in the DVE's pre-stream idle window.
Per-core: 22 key DMAs on the sync HWDGE ring, [p, (j d)] layout with
s = 1024g + 8p + j so each partition reads 32 KiB contiguous; DVE runs
128 AMR score columns (saturated ~98%); ACT runs one Square+accum per
supertile. A warm-up AllReduce (garbage input, result unused) pays the
ncfw wakeup before AR1 needs it.
"""

import sys

import numpy as np

if "/opt/trn_rl_repo" not in sys.path:
    sys.path.insert(0, "/opt/trn_rl_repo")

import concourse.bacc as bacc
import concourse.bass as bass
import concourse.mybir as mybir
import concourse.tile as tile
from concourse.bass_utils import run_bass_kernel_spmd

B, S, D = 32, 4096, 1024
NCORES = 8
BPC = B // NCORES  # batches per core
P = 128            # s-tile partition size
NT = S // P        # s-tiles per batch (32)
NC_TILES = BPC * NT  # score columns per core (128)
PERTURB = 1e-15

F32 = mybir.dt.float32
I32 = mybir.dt.int32
ALU = mybir.AluOpType
ACTF = mybir.ActivationFunctionType

SUB = 8          # s-tiles per key super-tile
NG = NT // SUB   # super-tiles per batch (4)
NST = BPC * NG   # super-tiles per core (16)
KEY_BUFS = 4
AR1_ST = 12      # super-tiles covered by AR1 (0..11); AR2 covers 12..15


def build() -> bass.Bass:
    nc = bacc.Bacc(
        "TRN2", target_bir_lowering=False, debug=False, num_devices=NCORES
    )
    key_ext = nc.declare_dram_parameter("key", [BPC, S, D], F32, isOutput=False)
    q_ext = nc.declare_dram_parameter("query", [BPC, D], F32, isOutput=False)
    sl_ext = nc.declare_dram_parameter("seq_lens", [1, BPC], I32, isOutput=False)
    out_ext = nc.declare_dram_parameter("out", [BPC, S, 1], F32, isOutput=True)

    cc_in = nc.dram_tensor("cc_in", [1, 1], F32)
    cc_out = nc.dram_tensor("cc_out", [1, 1], F32, addr_space="Shared")
    cc_in2 = nc.dram_tensor("cc_in2", [1, 1], F32)
    cc_out2 = nc.dram_tensor("cc_out2", [1, 1], F32, addr_space="Shared")
    ccw_in = nc.dram_tensor("ccw_in", [1, 1], F32)
    ccw_out = nc.dram_tensor("ccw_out", [1, 1], F32, addr_space="Shared")

    key_ap = key_ext.ap()
    out_ap = out_ext.ap()

    with tile.TileContext(nc) as tc:
        with (
            tc.tile_pool(name="keys", bufs=KEY_BUFS) as kpool,
            tc.tile_pool(name="amr_scratch", bufs=4) as amrpool,
            tc.tile_pool(name="mm_psum", bufs=1, space="PSUM") as psmall,
            tc.tile_pool(name="persist", bufs=1) as pp,
        ):
            # ---- persistent constants / small tiles ----
            ones_full = pp.tile([P, P], F32)
            nc.vector.memset(ones_full[:, :], 1.0)
            ones_row = pp.tile([1, P], F32)
            nc.vector.memset(ones_row[:, :], 1.0)

            # warm-up collective FIRST on gpsimd: input is garbage DRAM,
            # result unused; pays the ncfw wakeup latency (~50 us) long
            # before AR1 fires.
            nc.gpsimd.collective_compute(
                "AllReduce",
                ALU.add,
                replica_groups=[list(range(NCORES))],
                ins=[ccw_in.ap().opt()],
                outs=[ccw_out.ap().opt()],
            )

            # s_idx[p, c=(g,j)] = SUB*P*g + SUB*p + j (gpsimd iota, early)
            s_idx_i = pp.tile([P, NT], I32)
            nc.gpsimd.iota(
                s_idx_i[:, :],
                pattern=[[SUB * P, NG], [1, SUB]],
                base=0,
                channel_multiplier=SUB,
            )
            s_idx = pp.tile([P, NT], F32)
            nc.vector.tensor_copy(out=s_idx[:, :], in_=s_idx_i[:, :])

            # sl + q loads (scalar ring) and broadcasts (gpsimd); q_rep[0]
            # is ready before the first AMR needs it at ~10.5 us.
            sl_i = pp.tile([1, BPC], I32)
            nc.scalar.dma_start(out=sl_i[:, :], in_=sl_ext.ap()[:, :])
            q_tiles = []
            for b in range(BPC):
                qr = pp.tile([P, D], F32, tag=f"qrep{b}")
                nc.scalar.dma_start(out=qr[0:1, :], in_=q_ext.ap()[b : b + 1, :])
                q_tiles.append(qr)
            sl_f = pp.tile([P, BPC], F32)
            nc.vector.tensor_copy(out=sl_f[0:1, :], in_=sl_i[:, :])
            nc.gpsimd.partition_broadcast(q_tiles[0][:, :], q_tiles[0][0:1, :])
            nc.gpsimd.partition_broadcast(sl_f[:, :], sl_f[0:1, :])
            for b in range(1, BPC):
                nc.gpsimd.partition_broadcast(q_tiles[b][:, :], q_tiles[b][0:1, :])
            q_rep = [q_tiles[b][:, :] for b in range(BPC)]

            masks_all = pp.tile([P, NC_TILES], F32)
            for b in range(BPC):
                nc.vector.tensor_scalar(
                    out=masks_all[:, b * NT : (b + 1) * NT],
                    in0=s_idx[:, :],
                    scalar1=sl_f[:, b : b + 1],
                    scalar2=None,
                    op0=ALU.is_lt,
                )

            # ---- key streaming: 22 DMAs on the sync HWDGE ring ----
            # st0 and st15 go in 4x 1 MiB chunks INTO a normal key-tag
            # tile (subtile deps let per-chunk consumers start early).
            scores = pp.tile([P, NC_TILES], F32)
            # ssq1 cols: st0 x4 chunks + st1..11 x1 each = 15
            ssq1 = pp.tile([P, 15], F32)
            # ssq2 cols: st12,13 x1 + st14 x4 + st15 x5 = 11
            ssq2 = pp.tile([P, 11], F32)

            def st_src(t):
                b, g = divmod(t, NG)
                return key_ap[
                    b, g * SUB * P : (g + 1) * SUB * P, :
                ].rearrange("(p j) d -> p j d", p=P)

            key_tiles = [None] * NST

            def load_chunked(t, jsplits=((0, 2), (2, 4), (4, 6), (6, 8))):
                kt = kpool.tile([P, SUB * D], F32, tag="key")
                src = st_src(t)
                for j0, j1 in jsplits:
                    nc.sync.dma_start(
                        out=kt[:, j0 * D : j1 * D].rearrange(
                            "p (j d) -> p j d", d=D
                        ),
                        in_=src[:, j0:j1, :],
                    )
                key_tiles[t] = kt

            def load_full(t):
                kt = kpool.tile([P, SUB * D], F32, tag="key")
                nc.sync.dma_start(
                    out=kt[:, :].rearrange("p (j d) -> p j d", d=D),
                    in_=st_src(t),
                )
                key_tiles[t] = kt

            def amr_cols(t, js):
                b = t // NG
                kt = key_tiles[t]
                for j in js:
                    c = b * NT + (t % NG) * SUB + j
                    amr = amrpool.tile([P, D], F32, tag="amr")
                    nc.vector.affine_mul_reduce(
                        out=amr[:, :],
                        accum_out=scores[:, c : c + 1],
                        in0=kt[:, j * D : (j + 1) * D],
                        in1=q_rep[b][:, :],
                        scale=1.0,
                        bias=0.0,
                    )

            BF16 = mybir.dt.bfloat16

            def sqdump(w):
                # write-only dump for ACT Square (only accum_out matters)
                sqd = amrpool.tile(
                    [P, SUB * D], BF16, tag="sqd", bufs=1, name="sqd"
                )
                return sqd[:, 0:w]

            def sq_chunks(t, acc_tile, acc0, jsplits=((0, 2), (2, 4), (4, 6), (6, 8))):
                # one Square+accum per loaded chunk (pipelines with arrival)
                for i, (j0, j1) in enumerate(jsplits):
                    nc.scalar.activation(
                        out=sqdump((j1 - j0) * D),
                        in_=key_tiles[t][:, j0 * D : j1 * D],
                        func=ACTF.Square,
                        accum_out=acc_tile[:, acc0 + i : acc0 + i + 1],
                    )

            # --- emission: DMAs + compute interleaved per supertile ---
            # st0 chunked (early DVE start), st1..13 full 4 MiB (best DMA
            # rate; their FD=8192 squares trail arrival by ~7 us, not on
            # any critical path), st14/15 chunked (AR2 gate pipelines).
            load_chunked(0)
            sq_chunks(0, ssq1, 0)
            amr_cols(0, range(SUB))
            for t in range(1, AR1_ST):
                load_full(t)
                nc.scalar.activation(
                    out=sqdump(SUB * D),
                    in_=key_tiles[t][:, :],
                    func=ACTF.Square,
                    accum_out=ssq1[:, 3 + t : 4 + t],
                )
                amr_cols(t, range(SUB))

            # AR1 input: PE partition-sum -> ACT column-sum -> DMA.
            # No DVE involvement; fires as soon as sq13 lands (~77% of
            # the stream), absorbing inter-core skew under the stream.
            ps1 = psmall.tile([P, 15], F32, tag="ps1")
            nc.tensor.matmul(
                ps1[:, :], ones_full[:, :], ssq1[:, :], start=True, stop=True
            )
            g1_sb = pp.tile([1, 1], F32)
            csd1 = pp.tile([1, 15], F32, tag="csd1", name="csd1")
            nc.scalar.activation(
                out=csd1[:, :],
                in_=ps1[0:1, :],
                func=ACTF.Copy,
                accum_out=g1_sb[:, :],
            )
            nc.scalar.dma_start(out=cc_in.ap()[:, :], in_=g1_sb[:, :])
            nc.gpsimd.collective_compute(
                "AllReduce",
                ALU.add,
                replica_groups=[list(range(NCORES))],
                ins=[cc_in.ap().opt()],
                outs=[cc_out.ap().opt()],
            )

            # supertiles 12..15 (AR2); st12/13 full, st14/15 chunked; the
            # last chunk is 512 KB so its square (the AR2-input gate)
            # finishes ~1.4 us after the last byte
            for t in (12, 13):
                load_full(t)
                nc.scalar.activation(
                    out=sqdump(SUB * D),
                    in_=key_tiles[t][:, :],
                    func=ACTF.Square,
                    accum_out=ssq2[:, t - 12 : t - 11],
                )
                amr_cols(t, range(SUB))

            load_chunked(14)
            sq_chunks(14, ssq2, 2)
            amr_cols(14, range(SUB))

            LASTSPLIT = ((0, 2), (2, 4), (4, 6), (6, 7), (7, 8))
            load_chunked(NST - 1, LASTSPLIT)
            sq_chunks(NST - 1, ssq2, 6, LASTSPLIT)
            amr_cols(NST - 1, range(SUB))

            # AR2 input: same PE+ACT path, ready ~2.5 us after last byte.
            ps2 = psmall.tile([P, 11], F32, tag="ps2")
            nc.tensor.matmul(
                ps2[:, :], ones_full[:, :], ssq2[:, :], start=True, stop=True
            )
            g2_sb = pp.tile([1, 1], F32)
            csd2 = pp.tile([1, 11], F32, tag="csd2", name="csd2")
            nc.scalar.activation(
                out=csd2[:, :],
                in_=ps2[0:1, :],
                func=ACTF.Copy,
                accum_out=g2_sb[:, :],
            )
            nc.scalar.dma_start(out=cc_in2.ap()[:, :], in_=g2_sb[:, :])
            nc.gpsimd.collective_compute(
                "AllReduce",
                ALU.add,
                replica_groups=[list(range(NCORES))],
                ins=[cc_in2.ap().opt()],
                outs=[cc_out2.ap().opt()],
            )

            # ---- pre-AR2 epilogue on the AR1-only scale ----
            # g1 arrives mid-stream; sqrt on ACT (default table), recip on
            # DVE. E1 = exp(scores * inv1) as soon as the last AMR lands.
            g1a_sb = pp.tile([1, 1], F32)
            nc.sync.dma_start(out=g1a_sb[:, :], in_=cc_out.ap()[:, :])
            g1b = psmall.tile([P, 1], F32, tag="g1b")
            nc.tensor.matmul(
                g1b[:, :], ones_row[:, :], g1a_sb[:, :], start=True, stop=True
            )
            # inv1 = rsqrt(g1): bit-trick seed + 2 Newton steps, DVE-only
            # (no ACT table loads on the tail path)
            g1r = pp.tile([P, 1], F32)
            nc.vector.tensor_copy(out=g1r[:, :], in_=g1b[:, :])
            magic = pp.tile([P, 1], I32)
            nc.vector.memset(magic[:, :], 0x5F3759DF)
            halfbits = pp.tile([P, 1], I32)
            nc.vector.tensor_scalar(
                out=halfbits[:, :], in0=g1r[:, :].bitcast(I32), scalar1=1,
                scalar2=None, op0=ALU.logical_shift_right,
            )
            y_i = pp.tile([P, 1], I32)
            nc.vector.scalar_tensor_tensor(
                out=y_i[:, :], in0=magic[:, :], scalar=1, in1=halfbits[:, :],
                op0=ALU.mult, op1=ALU.subtract,
            )
            y = y_i[:, :].bitcast(F32)
            ya = pp.tile([P, 1], F32)
            yb = pp.tile([P, 1], F32)
            inv1 = pp.tile([P, 1], F32)
            for it, (src_ap, dst) in enumerate(
                [(y, ya[:, :]), (ya[:, :], inv1[:, :])]
            ):
                gy2 = pp.tile([P, 1], F32, tag=f"gy2_{it}", name=f"gy2_{it}")
                nc.vector.scalar_tensor_tensor(
                    out=gy2[:, :], in0=src_ap, scalar=g1r[:, 0:1], in1=src_ap,
                    op0=ALU.mult, op1=ALU.mult,
                )
                nc.vector.tensor_scalar(
                    out=yb[:, :], in0=gy2[:, :], scalar1=-0.5, scalar2=1.5,
                    op0=ALU.mult, op1=ALU.add,
                )
                nc.vector.tensor_scalar(
                    out=dst, in0=src_ap, scalar1=yb[:, 0:1], scalar2=None,
                    op0=ALU.mult,
                )
            invg1 = pp.tile([P, 1], F32)  # 1/g1 = inv1^2, for r = g2/g1
            nc.vector.tensor_scalar(
                out=invg1[:, :], in0=inv1[:, :], scalar1=inv1[:, 0:1],
                scalar2=None, op0=ALU.mult,
            )

            e1 = pp.tile([P, NC_TILES], F32)
            nc.scalar.activation(
                out=e1[:, :], in_=scores[:, :], func=ACTF.Exp, scale=inv1[:, :]
            )
            w_t = pp.tile([P, NC_TILES], F32)
            nc.vector.tensor_scalar(
                out=w_t[:, :], in0=scores[:, :], scalar1=inv1[:, 0:1],
                scalar2=None, op0=ALU.mult,
            )
            em = pp.tile([P, NC_TILES], F32)
            nc.vector.tensor_tensor(
                out=em[:, :], in0=e1[:, :], in1=masks_all[:, :], op=ALU.mult
            )
            fm = pp.tile([P, NC_TILES], F32)
            nc.vector.tensor_tensor(
                out=fm[:, :], in0=em[:, :], in1=w_t[:, :], op=ALU.mult
            )
            zc = pp.tile([P, 2 * BPC], F32)  # [ze(4) | zf(4)] column sums
            nc.vector.tensor_reduce(
                out=zc[:, 0:BPC].rearrange("p (b o) -> p b o", o=1),
                in_=em[:, :].rearrange("p (b t) -> p b t", t=NT),
                axis=mybir.AxisListType.X,
                op=ALU.add,
            )
            nc.vector.tensor_reduce(
                out=zc[:, BPC : 2 * BPC].rearrange("p (b o) -> p b o", o=1),
                in_=fm[:, :].rearrange("p (b t) -> p b t", t=NT),
                axis=mybir.AxisListType.X,
                op=ALU.add,
            )
            zs = psmall.tile([P, 2 * BPC], F32, tag="zs")
            nc.tensor.matmul(
                zs[:, :], ones_full[:, :], zc[:, :], start=True, stop=True
            )
            zs_sb = pp.tile([P, 2 * BPC], F32)
            nc.vector.tensor_copy(out=zs_sb[:, :], in_=zs[:, :])

            # ---- post-AR2: linearized correction, ~10 tiny ops ----
            g2a_sb = pp.tile([1, 1], F32)
            nc.sync.dma_start(out=g2a_sb[:, :], in_=cc_out2.ap()[:, :])
            g2b = psmall.tile([P, 1], F32, tag="g2b")
            nc.tensor.matmul(
                g2b[:, :], ones_row[:, :], g2a_sb[:, :], start=True, stop=True
            )
            # r = g2/g1 ; delta = r*(-1/2 + r*(3/8 - (5/16) r))
            r_t = pp.tile([P, 1], F32)
            nc.vector.tensor_scalar(
                out=r_t[:, :], in0=g2b[:, :], scalar1=invg1[:, 0:1],
                scalar2=None, op0=ALU.mult,
            )
            h_t = pp.tile([P, 1], F32)  # h = (35/128)*r - 5/16
            nc.vector.tensor_scalar(
                out=h_t[:, :], in0=r_t[:, :], scalar1=0.2734375,
                scalar2=-0.3125, op0=ALU.mult, op1=ALU.add,
            )
            h2_t = pp.tile([P, 1], F32)  # h2 = h*r + 3/8
            nc.vector.tensor_scalar(
                out=h2_t[:, :], in0=h_t[:, :], scalar1=r_t[:, 0:1],
                scalar2=0.375, op0=ALU.mult, op1=ALU.add,
            )
            h3_t = pp.tile([P, 1], F32)  # h3 = h2*r - 1/2
            nc.vector.tensor_scalar(
                out=h3_t[:, :], in0=h2_t[:, :], scalar1=r_t[:, 0:1],
                scalar2=-0.5, op0=ALU.mult, op1=ALU.add,
            )
            delta = pp.tile([P, 1], F32)  # delta = h3 * r
            nc.vector.tensor_scalar(
                out=delta[:, :], in0=h3_t[:, :], scalar1=r_t[:, 0:1],
                scalar2=None, op0=ALU.mult,
            )
            # Z = ze + delta*zf ; invz = 1/Z
            z_t = pp.tile([P, BPC], F32)
            nc.vector.scalar_tensor_tensor(
                out=z_t[:, :],
                in0=zs_sb[:, BPC : 2 * BPC],
                scalar=delta[:, 0:1],
                in1=zs_sb[:, 0:BPC],
                op0=ALU.mult,
                op1=ALU.add,
            )
            invz = pp.tile([P, BPC], F32)
            nc.vector.reciprocal(out=invz[:, :], in_=z_t[:, :])
            # u = em + delta*fm ; o = u*invz_b + PERTURB
            u_t = pp.tile([P, NC_TILES], F32)
            nc.vector.scalar_tensor_tensor(
                out=u_t[:, :],
                in0=fm[:, :],
                scalar=delta[:, 0:1],
                in1=em[:, :],
                op0=ALU.mult,
                op1=ALU.add,
            )
            o_all = pp.tile([P, NC_TILES], F32)
            for b in range(BPC):
                nc.vector.tensor_scalar(
                    out=o_all[:, b * NT : (b + 1) * NT],
                    in0=u_t[:, b * NT : (b + 1) * NT],
                    scalar1=invz[:, b : b + 1],
                    scalar2=PERTURB,
                    op0=ALU.mult,
                    op1=ALU.add,
                )
            dst = out_ap[:, :, 0].rearrange("b (g p j) -> p b g j", p=P, j=SUB)
            src = o_all[:, :].rearrange("p (b g j) -> p b g j", b=BPC, j=SUB)
            nc.sync.dma_start(out=dst, in_=src)

    nc.compile()
    return nc


_NC_CACHE = None


def _get_nc():
    global _NC_CACHE
    if _NC_CACHE is None:
        _NC_CACHE = build()
    return _NC_CACHE


def make_in_maps(key, query, seq_lens):
    key = np.ascontiguousarray(np.asarray(key, dtype=np.float32))
    query = np.ascontiguousarray(np.asarray(query, dtype=np.float32))
    seq_lens = np.ascontiguousarray(np.asarray(seq_lens, dtype=np.int32))
    in_maps = []
    for c in range(NCORES):
        lo, hi = c * BPC, (c + 1) * BPC
        in_maps.append(
            {
                "key": key[lo:hi],
                "query": query[lo:hi],
                "seq_lens": seq_lens[lo:hi].reshape(1, BPC),
            }
        )
    return in_maps


def kernel(key, query, seq_lens, **run_kwargs):
    nc = _get_nc()
    in_maps = make_in_maps(key, query, seq_lens)
    res = run_bass_kernel_spmd(
        nc, in_maps, core_ids=list(range(NCORES)), **run_kwargs
    )
    outs = [res.results[c]["out"].reshape(BPC, S, 1) for c in range(NCORES)]
    full = np.concatenate(outs, axis=0).astype(np.float32)
    if run_kwargs:
        kernel.last_results = res  # expose profile info to test harness
    return full
